# revision 6
# baseline (speedup 1.0000x reference)
"""Trainium2 Bass kernel: DarkChannelLoss (v2).

Computes -mean(dark_channel(x)) for x [32,3,512,512] f32, where
dark_channel = reflect-pad(7) -> min over channels -> 15x15 sliding-window
min (windows clipped at bottom/right, i.e. +inf padded by 14).

Sharding: pure data parallel over batch, 4 images per NeuronCore x 8 cores.
Each core computes per-partition partial sums of its dark-channel map; the
host combines them into the final scalar mean.

v2 changes vs v1 (105.7us):
  - R=8 rows per partition: one pair of images per tile generation
    (partitions 0..63 = image A rows 8p..8p+7, 64..127 = image B).
    Cast-DMA descriptors grow from 2KB to 16KB runs: SWDGE Q7 descriptor
    generation drops from ~36us to ~5us, and DVE op count drops 4x
    (fixed 151-cycle DVE init amortized over 4x bigger free dims).
  - dc min + sum fused into one tensor_tensor_reduce (the standalone
    in-place tensor_scalar accumulate ran at 1x DVE mode, 1.24us x8).
  - pass-2 processes all 4 images per W-tile (bi=4 free dim).
"""

import numpy as np

try:
    import concourse.bass as bass
except ImportError:  # pragma: no cover
    import sys

    sys.path.insert(0, "/opt/trn_rl_repo")
    import concourse.bass as bass

import concourse.mybir as mybir
import concourse.bacc as bacc
from concourse.tile import TileContext
from concourse.bass_utils import run_bass_kernel_spmd

F32 = mybir.dt.float32
BF16 = mybir.dt.bfloat16
INF = float("inf")
MIN = mybir.AluOpType.min
ADD = mybir.AluOpType.add

B, C, H, W = 32, 3, 512, 512
WIN = 15
PAD = WIN // 2          # 7
HP = H + 2 * PAD        # 526 padded rows
WP = W + 2 * PAD        # 526 padded cols
N_CORES = 8
N_IMG = B // N_CORES    # 4 images per core
R = 8                   # rows per partition
NPI = H // R            # 64 partitions per image
PT = (WP + 127) // 128  # 5 W tiles
FREE = PT * 128         # 640
MF = 544                # m tile free width (col = src_w + 8)
DEN = B * HP * WP

N2W = WP + WIN - 2      # 539
N4W = N2W - 2           # 537
N8W = N4W - 4           # 533


def build_program():
    nb = 2  # pairs of images
    bi = 2
    nc = bacc.Bacc("TRN2", target_bir_lowering=False, debug=False)
    x = nc.dram_tensor("x", [N_IMG, C, H, W], F32, kind="ExternalInput")
    out = nc.dram_tensor("out", [128, 1], F32, kind="ExternalOutput")

    n_acc = PT  # 4 full W-tiles + 1 packed

    with TileContext(nc) as tc:
        from contextlib import ExitStack

        with ExitStack() as ctx:
            constp = ctx.enter_context(tc.tile_pool(name="const", bufs=1))
            chp = ctx.enter_context(tc.tile_pool(name="ch", bufs=2))
            tmpp = ctx.enter_context(tc.tile_pool(name="tmp", bufs=2))
            mp = ctx.enter_context(tc.tile_pool(name="m", bufs=2))
            cascp = ctx.enter_context(tc.tile_pool(name="casc", bufs=1))
            rmp = ctx.enter_context(tc.tile_pool(name="rm", bufs=2))
            tbp = ctx.enter_context(tc.tile_pool(name="tb", bufs=1))
            hcp = ctx.enter_context(tc.tile_pool(name="hc", bufs=1))
            dcp = ctx.enter_context(tc.tile_pool(name="dc", bufs=1))
            accp = ctx.enter_context(tc.tile_pool(name="acc", bufs=1))
            psp = ctx.enter_context(tc.tile_pool(name="ps", bufs=1, space="PSUM"))

            ident = constp.tile([128, 128], BF16, tag="ident")
            idt = constp.tile([128, 128], mybir.dt.int16, tag="idt")
            nc.gpsimd.iota(idt[:, :], pattern=[[1, 128]], base=0, channel_multiplier=-1)
            nc.vector.tensor_single_scalar(
                ident[:, :], idt[:, :], 0, mybir.AluOpType.is_equal
            )
            acc = accp.tile([128, n_acc], F32, tag="acc")
            # packed tile for the narrow last W-tile of all 4 images:
            # image i sits at partitions 32i..32i+13; other lanes stay 0.0
            tbP4 = accp.tile([128, FREE], BF16, tag="tbP4")
            nc.vector.memset(tbP4[:, :], 0.0)

            # pass-2 input tiles, all 4 images: [w-col, img, padded-H]
            tb = [
                tbp.tile([128, N_IMG, FREE], BF16, tag=f"tb{p}", name=f"tb_{p}")
                for p in range(PT)
            ]
            for p in range(PT):
                nc.vector.memset(tb[p][:, :, WP:FREE], INF)

            pst = [
                psp.tile([128, R, 128], BF16, tag=f"pst{p}", name=f"pst_{p}")
                for p in range(PT)
            ]

            for b in range(nb):
                # ---- load: 4 cast DMAs (img x ch{0,1}/{2}), 16KB runs ----
                ch = chp.tile([128, C, R, W], BF16, tag="ch", name=f"ch_{b}")
                for ii in range(bi):
                    src = x[bi * b + ii].rearrange("c (p r) w -> p c r w", r=R)
                    dst = ch[64 * ii : 64 * (ii + 1)]
                    nc.gpsimd.dma_start(dst[:, 0:2], src[:, 0:2])
                    nc.gpsimd.dma_start(dst[:, 2:3], src[:, 2:3])

                # ---- channel min -> padded m tile ----
                m = mp.tile([128, R, MF], BF16, tag="m", name=f"m_{b}")
                nc.vector.memset(m[:, :, 8 + W : MF], INF)
                tmp = tmpp.tile([128, R, W], BF16, tag="tmp", name=f"tmp_{b}")
                nc.vector.tensor_tensor(tmp[:, :, :], ch[:, 0], ch[:, 1], MIN)
                nc.vector.tensor_tensor(
                    m[:, :, 8 : 8 + W], tmp[:, :, :], ch[:, 2], MIN
                )
                # reflect pads: padded 0..6 <- cols 15..9; 519..525 <- 518..512
                nc.scalar.copy(m[:, :, 1:8], m[:, :, 15:8:-1])
                nc.scalar.copy(m[:, :, 520:527], m[:, :, 518:511:-1])

                # ---- W-direction sliding-min cascade ----
                w2 = cascp.tile([128, R, MF], BF16, tag="w2", name=f"w2_{b}")
                w4 = cascp.tile([128, R, MF], BF16, tag="w4", name=f"w4_{b}")
                w8 = cascp.tile([128, R, MF], BF16, tag="w8", name=f"w8_{b}")
                nc.vector.tensor_tensor(
                    w2[:, :, 0:N2W], m[:, :, 1 : N2W + 1], m[:, :, 2 : N2W + 2], MIN
                )
                nc.vector.tensor_tensor(
                    w4[:, :, 0:N4W], w2[:, :, 0:N4W], w2[:, :, 2 : N4W + 2], MIN
                )
                nc.vector.tensor_tensor(
                    w8[:, :, 0:N8W], w4[:, :, 0:N8W], w4[:, :, 4 : N8W + 4], MIN
                )
                rm = rmp.tile([128, R, FREE], BF16, tag="rm", name=f"rm_{b}")
                nc.vector.memset(rm[:, :, WP:FREE], INF)
                nc.vector.tensor_tensor(
                    rm[:, :, 0:WP], w8[:, :, 0:WP], w8[:, :, PAD : WP + PAD], MIN
                )

                # ---- transpose 128x128 blocks into PSUM, evacuate ----
                for p in range(PT):
                    for r in range(R):
                        nc.tensor.transpose(
                            pst[p][:, r, :],
                            rm[:, r, 128 * p : 128 * (p + 1)],
                            ident[:, :],
                        )
                    for ii in range(bi):
                        nc.scalar.copy(
                            tb[p][:, bi * b + ii, PAD : PAD + H].rearrange(
                                "a (p r) -> a r p", r=R
                            ),
                            pst[p][:, :, 64 * ii : 64 * ii + 64],
                        )

            # ---- pass 2: H-direction cascade per W-tile, 4 images fused ----
            for p in range(PT):
                tbv = tb[p]
                # H reflect pads on the free dim
                nc.scalar.copy(tbv[:, :, 0:PAD], tbv[:, :, 2 * PAD : PAD : -1])
                nc.scalar.copy(
                    tbv[:, :, H + PAD : HP], tbv[:, :, H + PAD - 2 : H - 2 : -1]
                )
                if p == PT - 1:
                    for i in range(N_IMG):
                        nc.scalar.copy(
                            tbP4[32 * i : 32 * i + 14, :], tbv[0:14, i, :]
                        )
                    continue

                h2 = hcp.tile([128, N_IMG, MF], BF16, tag="h2", name=f"h2_{p}")
                h4 = hcp.tile([128, N_IMG, MF], BF16, tag="h4", name=f"h4_{p}")
                h8 = hcp.tile([128, N_IMG, MF], BF16, tag="h8", name=f"h8_{p}")
                nc.vector.tensor_tensor(
                    h2[:, :, 0:N2W], tbv[:, :, 0:N2W], tbv[:, :, 1 : N2W + 1], MIN
                )
                nc.vector.tensor_tensor(
                    h4[:, :, 0:N4W], h2[:, :, 0:N4W], h2[:, :, 2 : N4W + 2], MIN
                )
                nc.vector.tensor_tensor(
                    h8[:, :, 0:N8W], h4[:, :, 0:N8W], h4[:, :, 4 : N8W + 4], MIN
                )
                dc = dcp.tile([128, N_IMG, HP], BF16, tag="dc", name=f"dc_{p}")
                nc.vector.scalar_tensor_tensor(
                    dc[:, :, :],
                    h8[:, :, 0:HP],
                    0.0,
                    h8[:, :, PAD : HP + PAD],
                    mybir.AluOpType.bypass,
                    MIN,
                    accum_out=acc[:, p : p + 1],
                )

            # ---- packed last W-tile: one cascade for all 4 images ----
            g2 = cascp.tile([128, MF], BF16, tag="g2", name="g2")
            g4 = cascp.tile([128, MF], BF16, tag="g4", name="g4")
            g8 = cascp.tile([128, MF], BF16, tag="g8", name="g8")
            nc.vector.tensor_tensor(
                g2[:, 0:N2W], tbP4[:, 0:N2W], tbP4[:, 1 : N2W + 1], MIN
            )
            nc.vector.tensor_tensor(g4[:, 0:N4W], g2[:, 0:N4W], g2[:, 2 : N4W + 2], MIN)
            nc.vector.tensor_tensor(g8[:, 0:N8W], g4[:, 0:N8W], g4[:, 4 : N8W + 4], MIN)
            gdc = dcp.tile([128, HP], BF16, tag="gdc", name="gdc")
            nc.vector.scalar_tensor_tensor(
                gdc[:, :],
                g8[:, 0:HP],
                0.0,
                g8[:, PAD : HP + PAD],
                mybir.AluOpType.bypass,
                MIN,
                accum_out=acc[:, n_acc - 1 : n_acc],
            )

            tot = accp.tile([128, 1], F32, tag="tot")
            nc.vector.tensor_reduce(
                tot[:, 0:1],
                acc[:, 0:n_acc],
                axis=mybir.AxisListType.X,
                op=ADD,
            )
            nc.sync.dma_start(out[:, :], tot[:, :])

    return nc


_PROGRAM = None


def _get_program():
    global _PROGRAM
    if _PROGRAM is None:
        _PROGRAM = build_program()
        _PROGRAM.finalize()  # run Bacc passes (wait splitting, regalloc)
    return _PROGRAM


def kernel(generated_image):
    x = np.ascontiguousarray(np.asarray(generated_image), dtype=np.float32)
    assert x.shape == (B, C, H, W)
    nc = _get_program()
    shards = x.reshape(N_CORES, N_IMG, C, H, W)
    in_maps = [{"x": np.ascontiguousarray(shards[i])} for i in range(N_CORES)]
    res = run_bass_kernel_spmd(nc, in_maps, list(range(N_CORES)))
    total = float(np.sum([r["out"].astype(np.float64).sum() for r in res.results]))
    return np.array(-total / DEN, dtype=np.float32)


# revision 7
# speedup vs baseline: 1.0651x; 1.0651x over previous
"""Trainium2 Bass kernel: DarkChannelLoss (v3).

Computes -mean(dark_channel(x)) for x [32,3,512,512] f32, where
dark_channel = reflect-pad(7) -> min over channels -> 15x15 sliding-window
min (windows clipped at bottom/right, i.e. +inf padded by 14).

Sharding: pure data parallel over batch, 4 images per NeuronCore x 8 cores.
Each core computes per-partition partial sums of its dark-channel map; the
host combines them into the final scalar mean.

Structure (per core, 2 pairs of 2 images):
  - Row mapping h = 64*r + p: partition p (64 per image) holds rows
    {p, p+64, ..., p+448} as free-dim blocks r=0..7. Blocked so that
    TensorE-transpose output (free dim = p) lands CONTIGUOUSLY in the
    pass-2 tile for each r: evacuation copies are stride-1.
  - Loads: HWDGE f32 DMAs (RTL descriptor generation, no Q7 cost),
    ACT casts f32->bf16 into the channel tile.
  - Pass 1: channel-min then W-direction sliding-min cascade with free
    dim = 8 rows x ~540 cols (big ops amortize the DVE 151-cycle init).
  - TensorE 128x128 transposes into PSUM; ACT evacuates (contiguous).
  - Pass 2: H-direction cascade, all 4 images fused per W-tile; final
    min+sum fused via scalar_tensor_tensor accum_out.
  - Identity matrix for transposes is a host-provided input (gpsimd iota
    costs a 35us Q7 IRAM load that blocked all transposes).
"""

import numpy as np

try:
    import concourse.bass as bass
except ImportError:  # pragma: no cover
    import sys

    sys.path.insert(0, "/opt/trn_rl_repo")
    import concourse.bass as bass

import concourse.mybir as mybir
import concourse.bacc as bacc
from concourse.tile import TileContext
from concourse.bass_utils import run_bass_kernel_spmd

F32 = mybir.dt.float32
BF16 = mybir.dt.bfloat16
INF = float("inf")
MIN = mybir.AluOpType.min
ADD = mybir.AluOpType.add
BYPASS = mybir.AluOpType.bypass

B, C, H, W = 32, 3, 512, 512
WIN = 15
PAD = WIN // 2          # 7
HP = H + 2 * PAD        # 526 padded rows
WP = W + 2 * PAD        # 526 padded cols
N_CORES = 8
N_IMG = B // N_CORES    # 4 images per core
R = 8                   # row blocks per partition
NPI = H // R            # 64 partitions per image
PT = (WP + 127) // 128  # 5 W tiles
FREE = PT * 128         # 640
MF = 544                # m tile free width (col = src_w + 8)
DEN = B * HP * WP

N2W = WP + WIN - 2      # 539
N4W = N2W - 2           # 537
N8W = N4W - 4           # 533


def build_program():
    nb = 2  # pairs of images
    bi = 2
    nc = bacc.Bacc("TRN2", target_bir_lowering=False, debug=False)
    x = nc.dram_tensor("x", [N_IMG, C, H, W], F32, kind="ExternalInput")
    ident_in = nc.dram_tensor("ident_in", [128, 128], BF16, kind="ExternalInput")
    out = nc.dram_tensor("out", [128, 1], F32, kind="ExternalOutput")

    n_acc = PT  # 4 full W-tiles + 1 packed

    with TileContext(nc) as tc:
        from contextlib import ExitStack

        with ExitStack() as ctx:
            constp = ctx.enter_context(tc.tile_pool(name="const", bufs=1))
            cfp = ctx.enter_context(tc.tile_pool(name="cf", bufs=2))
            chp = ctx.enter_context(tc.tile_pool(name="ch", bufs=2))
            tmpp = ctx.enter_context(tc.tile_pool(name="tmp", bufs=1))
            mp = ctx.enter_context(tc.tile_pool(name="m", bufs=2))
            cascp = ctx.enter_context(tc.tile_pool(name="casc", bufs=1))
            rmp = ctx.enter_context(tc.tile_pool(name="rm", bufs=1))
            tbp = ctx.enter_context(tc.tile_pool(name="tb", bufs=1))
            hcp = ctx.enter_context(tc.tile_pool(name="hc", bufs=1))
            dcp = ctx.enter_context(tc.tile_pool(name="dc", bufs=1))
            accp = ctx.enter_context(tc.tile_pool(name="acc", bufs=1))
            psp = ctx.enter_context(tc.tile_pool(name="ps", bufs=1, space="PSUM"))

            ident = constp.tile([128, 128], BF16, tag="ident")
            nc.sync.dma_start(ident[:, :], ident_in[:, :])

            acc = accp.tile([128, n_acc], F32, tag="acc")
            # packed tile for the narrow last W-tile of all 4 images:
            # image i sits at partitions 32i..32i+13; other lanes stay 0.0
            tbP4 = accp.tile([128, FREE], BF16, tag="tbP4")
            nc.vector.memset(tbP4[:, :], 0.0)

            # pass-2 input tiles, all 4 images: [w-col, img, padded-H]
            tb = [
                tbp.tile([128, N_IMG, FREE], BF16, tag=f"tb{p}", name=f"tb_{p}")
                for p in range(PT)
            ]
            for p in range(PT):
                nc.vector.memset(tb[p][:, :, WP:FREE], INF)

            pst = [
                psp.tile([128, R, 128], BF16, tag=f"pst{p}", name=f"pst_{p}")
                for p in range(PT)
            ]

            for b in range(nb):
                # ---- load: HWDGE f32 DMAs per (img, channel); ACT casts ----
                ch = chp.tile([128, C, R, W], BF16, tag="ch", name=f"ch_{b}")
                cf = [
                    cfp.tile([128, R, W], F32, tag="cf", name=f"cf_{b}_{c}")
                    for c in range(C)
                ]
                for c in range(C):
                    for ii in range(bi):
                        src = x[bi * b + ii, c].rearrange("(r p) w -> p r w", p=NPI)
                        nc.sync.dma_start(cf[c][64 * ii : 64 * (ii + 1)], src)
                for c in range(C):
                    nc.scalar.copy(ch[:, c], cf[c][:, :, :])

                # ---- channel min -> padded m tile ----
                m = mp.tile([128, R, MF], BF16, tag="m", name=f"m_{b}")
                nc.vector.memset(m[:, :, 8 + W : MF], INF)
                tmp = tmpp.tile([128, R, W], BF16, tag="tmp", name=f"tmp_{b}")
                nc.vector.tensor_tensor(tmp[:, :, :], ch[:, 0], ch[:, 1], MIN)
                nc.vector.tensor_tensor(
                    m[:, :, 8 : 8 + W], tmp[:, :, :], ch[:, 2], MIN
                )
                # reflect pads: padded 0..6 <- cols 15..9; 519..525 <- 518..512
                nc.scalar.copy(m[:, :, 1:8], m[:, :, 15:8:-1])
                nc.scalar.copy(m[:, :, 520:527], m[:, :, 518:511:-1])

                # ---- W-direction sliding-min cascade ----
                w2 = cascp.tile([128, R, MF], BF16, tag="w2", name=f"w2_{b}")
                w4 = cascp.tile([128, R, MF], BF16, tag="w4", name=f"w4_{b}")
                w8 = cascp.tile([128, R, MF], BF16, tag="w8", name=f"w8_{b}")
                nc.vector.tensor_tensor(
                    w2[:, :, 0:N2W], m[:, :, 1 : N2W + 1], m[:, :, 2 : N2W + 2], MIN
                )
                nc.vector.tensor_tensor(
                    w4[:, :, 0:N4W], w2[:, :, 0:N4W], w2[:, :, 2 : N4W + 2], MIN
                )
                nc.vector.tensor_tensor(
                    w8[:, :, 0:N8W], w4[:, :, 0:N8W], w4[:, :, 4 : N8W + 4], MIN
                )
                rm = rmp.tile([128, R, FREE], BF16, tag="rm", name=f"rm_{b}")
                nc.vector.memset(rm[:, :, WP:FREE], INF)
                nc.vector.tensor_tensor(
                    rm[:, :, 0:WP], w8[:, :, 0:WP], w8[:, :, PAD : WP + PAD], MIN
                )

                # ---- transpose 128x128 blocks into PSUM, evacuate ----
                for p in range(PT):
                    for r in range(R):
                        nc.tensor.transpose(
                            pst[p][:, r, :],
                            rm[:, r, 128 * p : 128 * (p + 1)],
                            ident[:, :],
                        )
                    for ii in range(bi):
                        # h = 64*r + p_local: contiguous run per image
                        nc.scalar.copy(
                            tb[p][:, bi * b + ii, PAD : PAD + H].rearrange(
                                "a (r p) -> a r p", r=R
                            ),
                            pst[p][:, :, 64 * ii : 64 * ii + 64],
                        )

            # ---- pass 2: H-direction cascade per W-tile, 4 images fused ----
            for p in range(PT):
                tbv = tb[p]
                # H reflect pads on the free dim
                nc.scalar.copy(tbv[:, :, 0:PAD], tbv[:, :, 2 * PAD : PAD : -1])
                nc.scalar.copy(
                    tbv[:, :, H + PAD : HP], tbv[:, :, H + PAD - 2 : H - 2 : -1]
                )
                if p == PT - 1:
                    for i in range(N_IMG):
                        nc.scalar.copy(
                            tbP4[32 * i : 32 * i + 14, :], tbv[0:14, i, :]
                        )
                    continue

                h2 = hcp.tile([128, N_IMG, MF], BF16, tag="h2", name=f"h2_{p}")
                h4 = hcp.tile([128, N_IMG, MF], BF16, tag="h4", name=f"h4_{p}")
                h8 = hcp.tile([128, N_IMG, MF], BF16, tag="h8", name=f"h8_{p}")
                nc.vector.tensor_tensor(
                    h2[:, :, 0:N2W], tbv[:, :, 0:N2W], tbv[:, :, 1 : N2W + 1], MIN
                )
                nc.vector.tensor_tensor(
                    h4[:, :, 0:N4W], h2[:, :, 0:N4W], h2[:, :, 2 : N4W + 2], MIN
                )
                nc.vector.tensor_tensor(
                    h8[:, :, 0:N8W], h4[:, :, 0:N8W], h4[:, :, 4 : N8W + 4], MIN
                )
                dc = dcp.tile([128, N_IMG, HP], BF16, tag="dc", name=f"dc_{p}")
                nc.vector.scalar_tensor_tensor(
                    dc[:, :, :],
                    h8[:, :, 0:HP],
                    0.0,
                    h8[:, :, PAD : HP + PAD],
                    BYPASS,
                    MIN,
                    accum_out=acc[:, p : p + 1],
                )

            # ---- packed last W-tile: one cascade for all 4 images ----
            g2 = cascp.tile([128, MF], BF16, tag="g2", name="g2")
            g4 = cascp.tile([128, MF], BF16, tag="g4", name="g4")
            g8 = cascp.tile([128, MF], BF16, tag="g8", name="g8")
            nc.vector.tensor_tensor(
                g2[:, 0:N2W], tbP4[:, 0:N2W], tbP4[:, 1 : N2W + 1], MIN
            )
            nc.vector.tensor_tensor(g4[:, 0:N4W], g2[:, 0:N4W], g2[:, 2 : N4W + 2], MIN)
            nc.vector.tensor_tensor(g8[:, 0:N8W], g4[:, 0:N8W], g4[:, 4 : N8W + 4], MIN)
            gdc = dcp.tile([128, HP], BF16, tag="gdc", name="gdc")
            nc.vector.scalar_tensor_tensor(
                gdc[:, :],
                g8[:, 0:HP],
                0.0,
                g8[:, PAD : HP + PAD],
                BYPASS,
                MIN,
                accum_out=acc[:, n_acc - 1 : n_acc],
            )

            tot = accp.tile([128, 1], F32, tag="tot")
            nc.vector.tensor_reduce(
                tot[:, 0:1],
                acc[:, 0:n_acc],
                axis=mybir.AxisListType.X,
                op=ADD,
            )
            nc.sync.dma_start(out[:, :], tot[:, :])

    return nc


_PROGRAM = None


def _get_program():
    global _PROGRAM
    if _PROGRAM is None:
        _PROGRAM = build_program()
        _PROGRAM.finalize()  # run Bacc passes (wait splitting, regalloc)
    return _PROGRAM


def _make_ident():
    return np.eye(128, dtype=mybir.dt.np(BF16))


def kernel(generated_image):
    x = np.ascontiguousarray(np.asarray(generated_image), dtype=np.float32)
    assert x.shape == (B, C, H, W)
    nc = _get_program()
    shards = x.reshape(N_CORES, N_IMG, C, H, W)
    ident = _make_ident()
    in_maps = [
        {"x": np.ascontiguousarray(shards[i]), "ident_in": ident}
        for i in range(N_CORES)
    ]
    res = run_bass_kernel_spmd(nc, in_maps, list(range(N_CORES)))
    total = float(np.sum([r["out"].astype(np.float64).sum() for r in res.results]))
    return np.array(-total / DEN, dtype=np.float32)


# revision 9
# speedup vs baseline: 1.3718x; 1.2879x over previous
"""Trainium2 Bass kernel: DarkChannelLoss (v3).

Computes -mean(dark_channel(x)) for x [32,3,512,512] f32, where
dark_channel = reflect-pad(7) -> min over channels -> 15x15 sliding-window
min (windows clipped at bottom/right, i.e. +inf padded by 14).

Sharding: pure data parallel over batch, 4 images per NeuronCore x 8 cores.
Each core computes per-partition partial sums of its dark-channel map; the
host combines them into the final scalar mean.

Structure (per core, 2 pairs of 2 images):
  - Row mapping h = 64*r + p: partition p (64 per image) holds rows
    {p, p+64, ..., p+448} as free-dim blocks r=0..7. Blocked so that
    TensorE-transpose output (free dim = p) lands CONTIGUOUSLY in the
    pass-2 tile for each r: evacuation copies are stride-1.
  - Loads: HWDGE f32 DMAs (RTL descriptor generation, no Q7 cost),
    ACT casts f32->bf16 into the channel tile.
  - Pass 1: channel-min then W-direction sliding-min cascade with free
    dim = 8 rows x ~540 cols (big ops amortize the DVE 151-cycle init).
  - TensorE 128x128 transposes into PSUM; ACT evacuates (contiguous).
  - Pass 2: H-direction cascade, all 4 images fused per W-tile; final
    min+sum fused via scalar_tensor_tensor accum_out.
  - Identity matrix for transposes is a host-provided input (gpsimd iota
    costs a 35us Q7 IRAM load that blocked all transposes).
"""

import numpy as np

try:
    import concourse.bass as bass
except ImportError:  # pragma: no cover
    import sys

    sys.path.insert(0, "/opt/trn_rl_repo")
    import concourse.bass as bass

import concourse.mybir as mybir
import concourse.bacc as bacc
from concourse.tile import TileContext
from concourse.bass_utils import run_bass_kernel_spmd

F32 = mybir.dt.float32
BF16 = mybir.dt.bfloat16
INF = float("inf")
MIN = mybir.AluOpType.min
ADD = mybir.AluOpType.add
BYPASS = mybir.AluOpType.bypass

B, C, H, W = 32, 3, 512, 512
WIN = 15
PAD = WIN // 2          # 7
HP = H + 2 * PAD        # 526 padded rows
WP = W + 2 * PAD        # 526 padded cols
N_CORES = 8
N_IMG = B // N_CORES    # 4 images per core
R = 8                   # row blocks per partition
NPI = H // R            # 64 partitions per image
PT = (WP + 127) // 128  # 5 W tiles
FREE = PT * 128         # 640
MF = 544                # m tile free width (col = src_w + 8)
DEN = B * HP * WP

N2W = WP + WIN - 2      # 539
N4W = N2W - 2           # 537
N8W = N4W - 4           # 533


def build_program():
    nb = 2  # pairs of images
    bi = 2
    nc = bacc.Bacc("TRN2", target_bir_lowering=False, debug=False)
    x = nc.dram_tensor("x", [N_IMG, C, H, W], F32, kind="ExternalInput")
    ident_in = nc.dram_tensor("ident_in", [128, 128], BF16, kind="ExternalInput")
    out = nc.dram_tensor("out", [128, 1], F32, kind="ExternalOutput")

    n_acc = PT  # 4 full W-tiles + 1 packed

    with TileContext(nc) as tc:
        from contextlib import ExitStack

        with ExitStack() as ctx:
            constp = ctx.enter_context(tc.tile_pool(name="const", bufs=1))
            chp = ctx.enter_context(tc.tile_pool(name="ch", bufs=2))
            tmpp = ctx.enter_context(tc.tile_pool(name="tmp", bufs=1))
            mp = ctx.enter_context(tc.tile_pool(name="m", bufs=2))
            cascp = ctx.enter_context(tc.tile_pool(name="casc", bufs=1))
            rmp = ctx.enter_context(tc.tile_pool(name="rm", bufs=1))
            tbp = ctx.enter_context(tc.tile_pool(name="tb", bufs=1))
            hcp = ctx.enter_context(tc.tile_pool(name="hc", bufs=1))
            dcp = ctx.enter_context(tc.tile_pool(name="dc", bufs=1))
            accp = ctx.enter_context(tc.tile_pool(name="acc", bufs=1))
            psp = ctx.enter_context(tc.tile_pool(name="ps", bufs=1, space="PSUM"))

            ident = constp.tile([128, 128], BF16, tag="ident")
            nc.sync.dma_start(ident[:, :], ident_in[:, :])

            acc = accp.tile([128, n_acc], F32, tag="acc")
            # packed tile for the narrow last W-tile of all 4 images:
            # image i sits at partitions 32i..32i+13; other lanes stay 0.0
            tbP4 = accp.tile([128, FREE], BF16, tag="tbP4")
            nc.vector.memset(tbP4[:, :], 0.0)

            # pass-2 input tiles, all 4 images: [w-col, img, padded-H]
            tb = [
                tbp.tile([128, N_IMG, FREE], BF16, tag=f"tb{p}", name=f"tb_{p}")
                for p in range(PT)
            ]
            for p in range(PT):
                nc.vector.memset(tb[p][:, :, WP:FREE], INF)

            pst = [
                psp.tile([128, R, 128], BF16, tag=f"pst{p}", name=f"pst_{p}")
                for p in range(PT)
            ]

            for b in range(nb):
                # ---- load: SWDGE cast DMAs f32->bf16, h = 64r + p mapping ----
                # order: ch{0,1} of both images first so chmin starts early
                ch = chp.tile([128, C, R, W], BF16, tag="ch", name=f"ch_{b}")
                for ii in range(bi):
                    src = x[bi * b + ii].rearrange("c (r p) w -> p c r w", p=NPI)
                    nc.gpsimd.dma_start(ch[64 * ii : 64 * (ii + 1), 0:2], src[:, 0:2])
                for ii in range(bi):
                    src = x[bi * b + ii].rearrange("c (r p) w -> p c r w", p=NPI)
                    nc.gpsimd.dma_start(ch[64 * ii : 64 * (ii + 1), 2:3], src[:, 2:3])

                # ---- channel min -> padded m tile ----
                m = mp.tile([128, R, MF], BF16, tag="m", name=f"m_{b}")
                nc.vector.memset(m[:, :, 8 + W : MF], INF)
                tmp = tmpp.tile([128, R, W], BF16, tag="tmp", name=f"tmp_{b}")
                nc.vector.tensor_tensor(tmp[:, :, :], ch[:, 0], ch[:, 1], MIN)
                nc.vector.tensor_tensor(
                    m[:, :, 8 : 8 + W], tmp[:, :, :], ch[:, 2], MIN
                )
                # reflect pads: padded 0..6 <- cols 15..9; 519..525 <- 518..512
                nc.scalar.copy(m[:, :, 1:8], m[:, :, 15:8:-1])
                nc.scalar.copy(m[:, :, 520:527], m[:, :, 518:511:-1])

                # ---- W-direction sliding-min cascade ----
                w2 = cascp.tile([128, R, MF], BF16, tag="w2", name=f"w2_{b}")
                w4 = cascp.tile([128, R, MF], BF16, tag="w4", name=f"w4_{b}")
                w8 = cascp.tile([128, R, MF], BF16, tag="w8", name=f"w8_{b}")
                nc.vector.tensor_tensor(
                    w2[:, :, 0:N2W], m[:, :, 1 : N2W + 1], m[:, :, 2 : N2W + 2], MIN
                )
                nc.vector.tensor_tensor(
                    w4[:, :, 0:N4W], w2[:, :, 0:N4W], w2[:, :, 2 : N4W + 2], MIN
                )
                nc.vector.tensor_tensor(
                    w8[:, :, 0:N8W], w4[:, :, 0:N8W], w4[:, :, 4 : N8W + 4], MIN
                )
                rm = rmp.tile([128, R, FREE], BF16, tag="rm", name=f"rm_{b}")
                nc.vector.memset(rm[:, :, WP:FREE], INF)
                nc.vector.tensor_tensor(
                    rm[:, :, 0:WP], w8[:, :, 0:WP], w8[:, :, PAD : WP + PAD], MIN
                )

                # ---- transpose 128x128 blocks into PSUM, evacuate ----
                for p in range(PT):
                    for r in range(R):
                        nc.tensor.transpose(
                            pst[p][:, r, :],
                            rm[:, r, 128 * p : 128 * (p + 1)],
                            ident[:, :],
                        )
                    for ii in range(bi):
                        # h = 64*r + p_local: contiguous run per image
                        nc.scalar.copy(
                            tb[p][:, bi * b + ii, PAD : PAD + H].rearrange(
                                "a (r p) -> a r p", r=R
                            ),
                            pst[p][:, :, 64 * ii : 64 * ii + 64],
                        )

            # ---- pass 2: H-direction cascade per W-tile, 4 images fused ----
            for p in range(PT):
                tbv = tb[p]
                # H reflect pads on the free dim
                nc.scalar.copy(tbv[:, :, 0:PAD], tbv[:, :, 2 * PAD : PAD : -1])
                nc.scalar.copy(
                    tbv[:, :, H + PAD : HP], tbv[:, :, H + PAD - 2 : H - 2 : -1]
                )
                if p == PT - 1:
                    for i in range(N_IMG):
                        nc.scalar.copy(
                            tbP4[32 * i : 32 * i + 14, :], tbv[0:14, i, :]
                        )
                    continue

                h2 = hcp.tile([128, N_IMG, MF], BF16, tag="h2", name=f"h2_{p}")
                h4 = hcp.tile([128, N_IMG, MF], BF16, tag="h4", name=f"h4_{p}")
                h8 = hcp.tile([128, N_IMG, MF], BF16, tag="h8", name=f"h8_{p}")
                nc.vector.tensor_tensor(
                    h2[:, :, 0:N2W], tbv[:, :, 0:N2W], tbv[:, :, 1 : N2W + 1], MIN
                )
                nc.vector.tensor_tensor(
                    h4[:, :, 0:N4W], h2[:, :, 0:N4W], h2[:, :, 2 : N4W + 2], MIN
                )
                nc.vector.tensor_tensor(
                    h8[:, :, 0:N8W], h4[:, :, 0:N8W], h4[:, :, 4 : N8W + 4], MIN
                )
                dc = dcp.tile([128, N_IMG, HP], BF16, tag="dc", name=f"dc_{p}")
                nc.vector.scalar_tensor_tensor(
                    dc[:, :, :],
                    h8[:, :, 0:HP],
                    0.0,
                    h8[:, :, PAD : HP + PAD],
                    BYPASS,
                    MIN,
                    accum_out=acc[:, p : p + 1],
                )

            # ---- packed last W-tile: one cascade for all 4 images ----
            g2 = cascp.tile([128, MF], BF16, tag="g2", name="g2")
            g4 = cascp.tile([128, MF], BF16, tag="g4", name="g4")
            g8 = cascp.tile([128, MF], BF16, tag="g8", name="g8")
            nc.vector.tensor_tensor(
                g2[:, 0:N2W], tbP4[:, 0:N2W], tbP4[:, 1 : N2W + 1], MIN
            )
            nc.vector.tensor_tensor(g4[:, 0:N4W], g2[:, 0:N4W], g2[:, 2 : N4W + 2], MIN)
            nc.vector.tensor_tensor(g8[:, 0:N8W], g4[:, 0:N8W], g4[:, 4 : N8W + 4], MIN)
            gdc = dcp.tile([128, HP], BF16, tag="gdc", name="gdc")
            nc.vector.scalar_tensor_tensor(
                gdc[:, :],
                g8[:, 0:HP],
                0.0,
                g8[:, PAD : HP + PAD],
                BYPASS,
                MIN,
                accum_out=acc[:, n_acc - 1 : n_acc],
            )

            tot = accp.tile([128, 1], F32, tag="tot")
            nc.vector.tensor_reduce(
                tot[:, 0:1],
                acc[:, 0:n_acc],
                axis=mybir.AxisListType.X,
                op=ADD,
            )
            nc.sync.dma_start(out[:, :], tot[:, :])

    return nc


_PROGRAM = None


def _get_program():
    global _PROGRAM
    if _PROGRAM is None:
        _PROGRAM = build_program()
        _PROGRAM.finalize()  # run Bacc passes (wait splitting, regalloc)
    return _PROGRAM


def _make_ident():
    return np.eye(128, dtype=mybir.dt.np(BF16))


def kernel(generated_image):
    x = np.ascontiguousarray(np.asarray(generated_image), dtype=np.float32)
    assert x.shape == (B, C, H, W)
    nc = _get_program()
    shards = x.reshape(N_CORES, N_IMG, C, H, W)
    ident = _make_ident()
    in_maps = [
        {"x": np.ascontiguousarray(shards[i]), "ident_in": ident}
        for i in range(N_CORES)
    ]
    res = run_bass_kernel_spmd(nc, in_maps, list(range(N_CORES)))
    total = float(np.sum([r["out"].astype(np.float64).sum() for r in res.results]))
    return np.array(-total / DEN, dtype=np.float32)


# revision 13
# speedup vs baseline: 1.5095x; 1.1004x over previous
"""Trainium2 Bass kernel: DarkChannelLoss (v3).

Computes -mean(dark_channel(x)) for x [32,3,512,512] f32, where
dark_channel = reflect-pad(7) -> min over channels -> 15x15 sliding-window
min (windows clipped at bottom/right, i.e. +inf padded by 14).

Sharding: pure data parallel over batch, 4 images per NeuronCore x 8 cores.
Each core computes per-partition partial sums of its dark-channel map; the
host combines them into the final scalar mean.

Structure (per core, 2 pairs of 2 images):
  - Row mapping h = 64*r + p: partition p (64 per image) holds rows
    {p, p+64, ..., p+448} as free-dim blocks r=0..7. Blocked so that
    TensorE-transpose output (free dim = p) lands CONTIGUOUSLY in the
    pass-2 tile for each r: evacuation copies are stride-1.
  - Loads: HWDGE f32 DMAs (RTL descriptor generation, no Q7 cost),
    ACT casts f32->bf16 into the channel tile.
  - Pass 1: channel-min then W-direction sliding-min cascade with free
    dim = 8 rows x ~540 cols (big ops amortize the DVE 151-cycle init).
  - TensorE 128x128 transposes into PSUM; ACT evacuates (contiguous).
  - Pass 2: H-direction cascade, all 4 images fused per W-tile; final
    min+sum fused via scalar_tensor_tensor accum_out.
  - Identity matrix for transposes is a host-provided input (gpsimd iota
    costs a 35us Q7 IRAM load that blocked all transposes).
"""

import numpy as np

try:
    import concourse.bass as bass
except ImportError:  # pragma: no cover
    import sys

    sys.path.insert(0, "/opt/trn_rl_repo")
    import concourse.bass as bass

import concourse.mybir as mybir
import concourse.bacc as bacc
from concourse.tile import TileContext
from concourse.bass_utils import run_bass_kernel_spmd

F32 = mybir.dt.float32
BF16 = mybir.dt.bfloat16
INF = float("inf")
MIN = mybir.AluOpType.min
ADD = mybir.AluOpType.add
BYPASS = mybir.AluOpType.bypass

B, C, H, W = 32, 3, 512, 512
WIN = 15
PAD = WIN // 2          # 7
HP = H + 2 * PAD        # 526 padded rows
WP = W + 2 * PAD        # 526 padded cols
N_CORES = 8
N_IMG = B // N_CORES    # 4 images per core
R = 8                   # row blocks per partition
NPI = H // R            # 64 partitions per image
PT = (WP + 127) // 128  # 5 W tiles
FREE = PT * 128         # 640
MF = 544                # m tile free width (col = src_w + 8)
DEN = B * HP * WP

N2W = WP + WIN - 2      # 539
N4W = N2W - 2           # 537
N8W = N4W - 4           # 533


def build_program():
    nb = 2  # pairs of images
    bi = 2
    nc = bacc.Bacc("TRN2", target_bir_lowering=False, debug=False)
    x = nc.dram_tensor("x", [N_IMG, C, H, W], F32, kind="ExternalInput")
    ident_in = nc.dram_tensor("ident_in", [128, 128], BF16, kind="ExternalInput")
    # all 128 per-partition partial sums, packed onto 4 partitions by a
    # 32x32 DVE block-transpose so the out-DMA is 4 contiguous descriptors
    # (a [128,1] out would scatter 128 4-byte descriptors, ~7us)
    out = nc.dram_tensor("out", [4, 32], F32, kind="ExternalOutput")

    n_acc = PT  # 4 full W-tiles + 1 packed

    with TileContext(nc) as tc:
        from contextlib import ExitStack

        with ExitStack() as ctx:
            constp = ctx.enter_context(tc.tile_pool(name="const", bufs=1))
            chp = ctx.enter_context(tc.tile_pool(name="ch", bufs=2))
            tmpp = ctx.enter_context(tc.tile_pool(name="tmp", bufs=1))
            mp = ctx.enter_context(tc.tile_pool(name="m", bufs=2))
            cascp = ctx.enter_context(tc.tile_pool(name="casc", bufs=1))
            rmp = ctx.enter_context(tc.tile_pool(name="rm", bufs=1))
            tbp = ctx.enter_context(tc.tile_pool(name="tb", bufs=1))
            hcp = ctx.enter_context(tc.tile_pool(name="hc", bufs=1))
            dcp = ctx.enter_context(tc.tile_pool(name="dc", bufs=1))
            accp = ctx.enter_context(tc.tile_pool(name="acc", bufs=1))
            psp = ctx.enter_context(tc.tile_pool(name="ps", bufs=1, space="PSUM"))

            ident = constp.tile([128, 128], BF16, tag="ident")
            nc.sync.dma_start(ident[:, :], ident_in[:, :])

            acc = accp.tile([128, n_acc], F32, tag="acc")
            # packed tile for the narrow last W-tile of all 4 images:
            # image i sits at partitions 32i..32i+13; other lanes stay 0.0
            tbP4 = accp.tile([128, FREE], BF16, tag="tbP4")
            nc.vector.memset(tbP4[:, :], 0.0)

            # pass-2 input tiles, all 4 images: [w-col, img, padded-H]
            tb = [
                tbp.tile([128, N_IMG, FREE], BF16, tag=f"tb{p}", name=f"tb_{p}")
                for p in range(PT)
            ]
            for p in range(PT):
                nc.vector.memset(tb[p][:, :, WP:FREE], INF)

            pst = [
                psp.tile([128, R, 128], BF16, tag=f"pst{p}", name=f"pst_{p}")
                for p in range(PT)
            ]

            for b in range(nb):
                # ---- load: SWDGE cast DMAs f32->bf16, h = 64r + p mapping ----
                # order: ch{0,1} of both images first so chmin starts early
                ch = chp.tile([128, C, R, W], BF16, tag="ch", name=f"ch_{b}")
                for ii in range(bi):
                    src = x[bi * b + ii].rearrange("c (r p) w -> p c r w", p=NPI)
                    nc.gpsimd.dma_start(ch[64 * ii : 64 * (ii + 1), 0:2], src[:, 0:2])
                for ii in range(bi):
                    src = x[bi * b + ii].rearrange("c (r p) w -> p c r w", p=NPI)
                    nc.gpsimd.dma_start(ch[64 * ii : 64 * (ii + 1), 2:3], src[:, 2:3])

                # ---- channel min -> padded m tile ----
                m = mp.tile([128, R, MF], BF16, tag="m", name=f"m_{b}")
                nc.vector.memset(m[:, :, 8 + W : MF], INF)
                tmp = tmpp.tile([128, R, W], BF16, tag="tmp", name=f"tmp_{b}")
                nc.vector.tensor_tensor(tmp[:, :, :], ch[:, 0], ch[:, 1], MIN)
                nc.vector.tensor_tensor(
                    m[:, :, 8 : 8 + W], tmp[:, :, :], ch[:, 2], MIN
                )
                # reflect pads: padded 0..6 <- cols 15..9; 519..525 <- 518..512
                nc.scalar.copy(m[:, :, 1:8], m[:, :, 15:8:-1])
                nc.scalar.copy(m[:, :, 520:527], m[:, :, 518:511:-1])

                # ---- W-direction sliding-min cascade ----
                w2 = cascp.tile([128, R, MF], BF16, tag="w2", name=f"w2_{b}")
                w4 = cascp.tile([128, R, MF], BF16, tag="w4", name=f"w4_{b}")
                w8 = cascp.tile([128, R, MF], BF16, tag="w8", name=f"w8_{b}")
                nc.vector.tensor_tensor(
                    w2[:, :, 0:N2W], m[:, :, 1 : N2W + 1], m[:, :, 2 : N2W + 2], MIN
                )
                nc.vector.tensor_tensor(
                    w4[:, :, 0:N4W], w2[:, :, 0:N4W], w2[:, :, 2 : N4W + 2], MIN
                )
                nc.vector.tensor_tensor(
                    w8[:, :, 0:N8W], w4[:, :, 0:N8W], w4[:, :, 4 : N8W + 4], MIN
                )
                rm = rmp.tile([128, R, FREE], BF16, tag="rm", name=f"rm_{b}")
                nc.vector.memset(rm[:, :, WP:FREE], INF)
                nc.vector.tensor_tensor(
                    rm[:, :, 0:WP], w8[:, :, 0:WP], w8[:, :, PAD : WP + PAD], MIN
                )

                # ---- transpose 128x128 blocks into PSUM, evacuate ----
                for p in range(PT):
                    for r in range(R):
                        nc.tensor.transpose(
                            pst[p][:, r, :],
                            rm[:, r, 128 * p : 128 * (p + 1)],
                            ident[:, :],
                        )
                    for ii in range(bi):
                        # h = 64*r + p_local: contiguous run per image
                        nc.scalar.copy(
                            tb[p][:, bi * b + ii, PAD : PAD + H].rearrange(
                                "a (r p) -> a r p", r=R
                            ),
                            pst[p][:, :, 64 * ii : 64 * ii + 64],
                        )
                    if b == nb - 1:
                        # both pairs evacuated: H reflect pads for this W-tile
                        tbv = tb[p]
                        nc.scalar.copy(
                            tbv[:, :, 0:PAD], tbv[:, :, 2 * PAD : PAD : -1]
                        )
                        nc.scalar.copy(
                            tbv[:, :, H + PAD : HP],
                            tbv[:, :, H + PAD - 2 : H - 2 : -1],
                        )

            # ---- pass 2: H-direction cascade per W-tile, 4 images fused ----
            for p in range(PT):
                tbv = tb[p]
                if p == PT - 1:
                    for i in range(N_IMG):
                        nc.scalar.copy(
                            tbP4[32 * i : 32 * i + 14, :], tbv[0:14, i, :]
                        )
                    continue

                h2 = hcp.tile([128, N_IMG, MF], BF16, tag="h2", name=f"h2_{p}")
                h4 = hcp.tile([128, N_IMG, MF], BF16, tag="h4", name=f"h4_{p}")
                h8 = hcp.tile([128, N_IMG, MF], BF16, tag="h8", name=f"h8_{p}")
                nc.vector.tensor_tensor(
                    h2[:, :, 0:N2W], tbv[:, :, 0:N2W], tbv[:, :, 1 : N2W + 1], MIN
                )
                nc.vector.tensor_tensor(
                    h4[:, :, 0:N4W], h2[:, :, 0:N4W], h2[:, :, 2 : N4W + 2], MIN
                )
                nc.vector.tensor_tensor(
                    h8[:, :, 0:N8W], h4[:, :, 0:N8W], h4[:, :, 4 : N8W + 4], MIN
                )
                dc = dcp.tile([128, N_IMG, HP], BF16, tag="dc", name=f"dc_{p}")
                nc.vector.scalar_tensor_tensor(
                    dc[:, :, :],
                    h8[:, :, 0:HP],
                    0.0,
                    h8[:, :, PAD : HP + PAD],
                    BYPASS,
                    MIN,
                    accum_out=acc[:, p : p + 1],
                )

            # ---- packed last W-tile: one cascade for all 4 images ----
            g2 = cascp.tile([128, MF], BF16, tag="g2", name="g2")
            g4 = cascp.tile([128, MF], BF16, tag="g4", name="g4")
            g8 = cascp.tile([128, MF], BF16, tag="g8", name="g8")
            nc.vector.tensor_tensor(
                g2[:, 0:N2W], tbP4[:, 0:N2W], tbP4[:, 1 : N2W + 1], MIN
            )
            nc.vector.tensor_tensor(g4[:, 0:N4W], g2[:, 0:N4W], g2[:, 2 : N4W + 2], MIN)
            nc.vector.tensor_tensor(g8[:, 0:N8W], g4[:, 0:N8W], g4[:, 4 : N8W + 4], MIN)
            gdc = dcp.tile([128, HP], BF16, tag="gdc", name="gdc")
            nc.vector.scalar_tensor_tensor(
                gdc[:, :],
                g8[:, 0:HP],
                0.0,
                g8[:, PAD : HP + PAD],
                BYPASS,
                MIN,
                accum_out=acc[:, n_acc - 1 : n_acc],
            )

            tot = accp.tile([128, 32], F32, tag="tot")
            nc.vector.tensor_reduce(
                tot[:, 0:1],
                acc[:, 0:n_acc],
                axis=mybir.AxisListType.X,
                op=ADD,
            )
            totT = accp.tile([128, 32], F32, tag="totT")
            nc.vector.transpose(totT[:, :], tot[:, :])
            nc.sync.dma_start(out[:, :], totT[0:128:32, 0:32])

    return nc


_PROGRAM = None


def _get_program():
    global _PROGRAM
    if _PROGRAM is None:
        _PROGRAM = build_program()
        _PROGRAM.finalize()  # run Bacc passes (wait splitting, regalloc)
    return _PROGRAM


def _make_ident():
    return np.eye(128, dtype=mybir.dt.np(BF16))


def kernel(generated_image):
    x = np.ascontiguousarray(np.asarray(generated_image), dtype=np.float32)
    assert x.shape == (B, C, H, W)
    nc = _get_program()
    shards = x.reshape(N_CORES, N_IMG, C, H, W)
    ident = _make_ident()
    in_maps = [
        {"x": np.ascontiguousarray(shards[i]), "ident_in": ident}
        for i in range(N_CORES)
    ]
    res = run_bass_kernel_spmd(nc, in_maps, list(range(N_CORES)))
    total = float(np.sum([r["out"].astype(np.float64).sum() for r in res.results]))
    return np.array(-total / DEN, dtype=np.float32)
